# revision 21
# baseline (speedup 1.0000x reference)
"""VQ dictionary-learning forward kernel for Trainium2 (8 NeuronCores).

Per 128-token tile (tokens on partitions):
  - logits = x @ rep_w.T            : fp16 matmul (smooth path)
  - s2     = x @ dict_w.T - w2/2    : fp16 hi/lo 3-term split => exact
                                      (~1e-6 abs; selection-critical),
                                      w2 folded in as a c=3 row-stack matmul
  - rep    = softmax over K=1024    : exp+rowsum on ACT, scale on DVE
  - top-8  : vector.max (max8) over s2 PSUM; threshold = 8th value
  - rep_sparse = (s2 >= t8) * rep/8 : one fused scalar_tensor_tensor
  - z_dl   = rep_sparse @ dict_w    : fp16 PE transposes + fp16 matmul
  - loss / perplexity / straight-through / NCHW: host side (scalar stats)
Sharding: data-parallel over 32768 tokens, 4096/core; weights replicated.
"""
import sys

sys.path.insert(0, '/opt/trn_rl_repo')

import numpy as np

DIM = 256
K = 1024
SP = 8
BETA = 0.25
EPS = 1e-10
B = 32
HW = 32
N = B * HW * HW          # 32768 tokens
NCORES = 8
NSH = N // NCORES        # 4096 tokens per core
NT = NSH // 128          # 32 tiles per core
import os as _os
_KNT = int(_os.environ.get("KNT", NT))
_SKIP_ZDL = _os.environ.get("SKIP_ZDL", "0") == "1"
_SKIP_W2 = _os.environ.get("SKIP_W2", "0") == "1"
_SKIP_MAXSTT = _os.environ.get("SKIP_MAXSTT", "0") == "1"

KB = K // 512            # 2 k-chunks of 512


def _split16(a):
    """fp32 -> (hi, lo) fp16 pair with hi + lo ~= a to ~2^-22."""
    hi = a.astype(np.float16)
    lo = (a - hi.astype(np.float32)).astype(np.float16)
    return np.ascontiguousarray(hi), np.ascontiguousarray(lo)


def build_nc():
    import concourse.bacc as bacc
    import concourse.mybir as mybir
    import concourse.tile as tile

    f32 = mybir.dt.float32
    f16 = mybir.dt.float16
    ACT = mybir.ActivationFunctionType
    ALU = mybir.AluOpType

    nc = bacc.Bacc(None, target_bir_lowering=False)

    # ---- DRAM I/O ----
    xhlT = nc.dram_tensor("xhlT", [2, DIM, NSH], f16, kind="ExternalInput")
    wstack = nc.dram_tensor("wstack", [3, 2, 128, K], f16, kind="ExternalInput")
    w2rep = nc.dram_tensor("w2rep", [128, K], f32, kind="ExternalInput")
    dw16 = nc.dram_tensor("dw16", [K, DIM], f16, kind="ExternalInput")
    ident = nc.dram_tensor("ident", [128, 128], f16, kind="ExternalInput")
    out_rs = nc.dram_tensor("out_rs", [NSH, K], f32, kind="ExternalOutput")
    out_zdl = nc.dram_tensor("out_zdl", [NSH, DIM], f32, kind="ExternalOutput")

    with tile.TileContext(nc) as tc:
        with (
            tc.tile_pool(name="wts", bufs=1) as wts,
            tc.tile_pool(name="io", bufs=3) as io,
            tc.tile_pool(name="wk", bufs=2) as wk,
            tc.tile_pool(name="pslg", bufs=1, space="PSUM") as pslg,
            tc.tile_pool(name="pss2", bufs=2, space="PSUM") as pss2,
            tc.tile_pool(name="pso", bufs=1, space="PSUM") as pso,
        ):
            # ---- prefetch x tiles 0/1 before weights on the sync queue ----
            xhl_pre = []
            for i in range(2):
                t = io.tile([128, 2, 2, 128], f16, name=f"xhl_{i}", tag="xhl")
                nc.sync.dma_start(
                    t[:], xhlT[:, :, i * 128:(i + 1) * 128].rearrange(
                        "hl (c p) t -> p hl c t", p=128))
                xhl_pre.append(t)

            # ---- persistent weights: one big stacked tile, 2 parallel DMAs ----
            W_sb = wts.tile([128, 3, 2, K], f16, name="W_sb", tag="W_sb")
            nc.sync.dma_start(
                W_sb[:, :, 0, :], wstack[:, 0].rearrange("w p k -> p w k"))
            nc.scalar.dma_start(
                W_sb[:, :, 1, :], wstack[:, 1].rearrange("w p k -> p w k"))
            wh_sb = [W_sb[:, 0, c] for c in range(2)]
            wl_sb = [W_sb[:, 1, c] for c in range(2)]
            rw_sb = [W_sb[:, 2, c] for c in range(2)]
            w2_sb = wts.tile([128, K], f32, name="w2sb", tag="w2sb")
            nc.scalar.dma_start(w2_sb[:], w2rep[:])
            id_sb = wts.tile([128, 128], f16, name="idsb", tag="idsb")
            nc.scalar.dma_start(id_sb[:], ident[:])
            dw_sb = wts.tile([128, 8, DIM], f16, name="dwsb", tag="dwsb")
            nc.scalar.dma_start(dw_sb[:], dw16[:].rearrange("(j p) d -> p j d", p=128))

            # ---- PE warm-up: HAM un-throttles after ~3.4us of activity.
            # These depend only on a memset, so they run during the initial
            # weight/x DMAs which would otherwise leave the PE cold.
            wu_sb = wts.tile([128, 512], f16, name="wu_sb", tag="wu")
            nc.vector.memset(wu_sb[:], 0.0)
            wu_ps = pso.tile([128, 512], f32, name="wu_ps", tag="tp")
            for wi in range(30):
                nc.tensor.matmul(wu_ps[:], wu_sb[:, 0:128], wu_sb[:],
                                 start=(wi == 0), stop=(wi == 29))

            def emit_transpose(pi, rs16_sb):
                rsT_sb = wk.tile([128, K], f16, name=f"rsT_{pi}", tag="rsT")
                tp_ps = pso.tile([128, K], f16, name=f"tp_{pi}", tag="tp")
                for jj in range(8):
                    nc.tensor.transpose(
                        tp_ps[:, jj * 128:(jj + 1) * 128],
                        rs16_sb[:, jj * 128:(jj + 1) * 128], id_sb[:])
                return (pi, rsT_sb, tp_ps)

            def emit_rst_copies(pi, rsT_sb, tp_ps):
                for half in range(2):
                    nc.vector.tensor_copy(rsT_sb[:, half * 512:(half + 1) * 512],
                                          tp_ps[:, half * 512:(half + 1) * 512])

            def emit_zdl(pi, rsT_sb, tp_ps):
                psel = slice(pi * 128, pi * 128 + 128)
                zdl_ps = pso.tile([128, DIM], f32, name=f"zdl_{pi}", tag="zdl")
                for j in range(8):
                    nc.tensor.matmul(zdl_ps[:], rsT_sb[:, j * 128:(j + 1) * 128],
                                     dw_sb[:, j, :],
                                     start=(j == 0), stop=(j == 7))
                zdl_sb = io.tile([128, DIM], f32, name=f"zdl_sb_{pi}", tag="zdlsb")
                nc.scalar.activation(zdl_sb[:], zdl_ps[:], ACT.Copy)
                nc.sync.dma_start(out_zdl[psel, :], zdl_sb[:])

            prev = None
            prev2 = None
            for i in range(_KNT):
                sel = slice(i * 128, i * 128 + 128)
                # ---- load x tile: one DMA for hi+lo, both 128-d chunks ----
                if i < 2:
                    xhl = xhl_pre[i]
                else:
                    xhl = io.tile([128, 2, 2, 128], f16, name=f"xhl_{i}", tag="xhl")
                    nc.sync.dma_start(
                        xhl[:], xhlT[:, :, sel].rearrange("hl (c p) t -> p hl c t", p=128))
                xh = xhl[:, 0]
                xl = xhl[:, 1]

                # ---- PE: logits first (unblocks ACT early), then s2 ----
                lg_ps = pslg.tile([128, K], f32, name=f"lg_{i}", tag="lg")
                s2_ps = pss2.tile([128, K], f32, name=f"s2_{i}", tag="s2")
                for kb in range(KB):
                    kbs = slice(kb * 512, kb * 512 + 512)
                    for c in range(2):
                        nc.tensor.matmul(lg_ps[:, kbs], xh[:, c], rw_sb[c][:, kbs],
                                         start=(c == 0), stop=(c == 1))
                for kb in range(KB):
                    kbs = slice(kb * 512, kb * 512 + 512)
                    for c in range(2):
                        nc.tensor.matmul(s2_ps[:, kbs], xh[:, c], wh_sb[c][:, kbs],
                                         start=(c == 0), stop=False)
                        nc.tensor.matmul(s2_ps[:, kbs], xh[:, c], wl_sb[c][:, kbs],
                                         start=False, stop=False)
                        nc.tensor.matmul(s2_ps[:, kbs], xl[:, c], wh_sb[c][:, kbs],
                                         start=False, stop=(c == 1))

                nc.vector.tensor_tensor(s2_ps[:], s2_ps[:], w2_sb[:],
                                        op=ALU.add)

                tstage = None
                if prev is not None:
                    tstage = emit_transpose(*prev)
                    prev = None
                if prev2 is not None:
                    emit_zdl(*prev2)
                    prev2 = None

                # ---- softmax + top-8 + fused sparse-mask ----
                e_sb = wk.tile([128, K], f32, name=f"e_{i}", tag="e")
                sum_sb = wk.tile([128, 1], f32, name=f"sum_{i}", tag="sum")
                nc.scalar.activation(e_sb[:], lg_ps[:], ACT.Exp,
                                     accum_out=sum_sb[:])
                rinv_sb = wk.tile([128, 1], f32, name=f"rinv_{i}", tag="rinv")
                nc.vector.reciprocal(rinv_sb[:], sum_sb[:])
                c8_sb = wk.tile([128, 1], f32, name=f"c8_{i}", tag="c8")
                nc.vector.tensor_scalar_mul(c8_sb[:], rinv_sb[:], 0.125)
                top8_sb = wk.tile([128, 8], f32, name=f"top8_{i}", tag="top8")
                rsraw_sb = wk.tile([128, K], f32, name=f"rsraw_{i}", tag="rsraw")
                nc.vector.max(top8_sb[:], s2_ps[:])
                nc.vector.scalar_tensor_tensor(
                    rsraw_sb[:], s2_ps[:], top8_sb[:, 7:8], e_sb[:],
                    op0=ALU.is_ge, op1=ALU.mult)
                rs_sb = io.tile([128, K], f32, name=f"rs_{i}", tag="rs")
                nc.scalar.activation(rs_sb[:], rsraw_sb[:], ACT.Copy,
                                     scale=c8_sb[:])
                nc.sync.dma_start(out_rs[sel, :], rs_sb[:])
                rs16_sb = wk.tile([128, K], f16, name=f"rs16_{i}", tag="rs16")
                nc.scalar.activation(rs16_sb[:], rsraw_sb[:], ACT.Copy,
                                     scale=c8_sb[:])
                prev = (i, rs16_sb)
                if tstage is not None:
                    emit_rst_copies(*tstage)
                    prev2 = tstage

            if prev2 is not None:
                emit_zdl(*prev2)
            for wi in range(18):
                nc.tensor.matmul(wu_ps[:], wu_sb[:, 0:128], wu_sb[:],
                                 start=(wi == 0), stop=(wi == 17),
                                 skip_group_check=True)
            if prev is not None:
                tstage = emit_transpose(*prev)
                emit_rst_copies(*tstage)
                emit_zdl(*tstage)

    nc.compile()
    return nc


_NC_CACHE = {}


def _get_nc():
    if "nc" not in _NC_CACHE:
        _NC_CACHE["nc"] = build_nc()
    return _NC_CACHE["nc"]


def prepare_in_maps(z_e, dict_w, rep_w, rep_b):
    z_e = np.asarray(z_e, dtype=np.float32)
    dict_w = np.asarray(dict_w, dtype=np.float32)
    rep_w = np.asarray(rep_w, dtype=np.float32)

    ze_flat = np.ascontiguousarray(
        np.transpose(z_e, (0, 2, 3, 1)).reshape(N, DIM))

    whT, wlT = _split16(np.ascontiguousarray(dict_w.T))
    rwT = np.ascontiguousarray(rep_w.T).astype(np.float16)
    wstack = np.ascontiguousarray(np.stack([whT, wlT, rwT]).reshape(3, 2, 128, K))
    # exact -w2/2 as a 3-term fp16 sum
    w2 = (dict_w.astype(np.float64) ** 2).sum(1)
    v = (-0.5 * w2).astype(np.float32)
    w2rep = np.ascontiguousarray(np.broadcast_to(v, (128, K)).copy())
    ident = np.eye(128, dtype=np.float16)
    dw16h = dict_w.astype(np.float16)

    in_maps = []
    for c in range(NCORES):
        xT = np.ascontiguousarray(ze_flat[c * NSH:(c + 1) * NSH].T)
        xhT, xlT_ = _split16(xT)
        xhl = np.ascontiguousarray(np.stack([xhT, xlT_]))
        in_maps.append({
            "xhlT": xhl,
            "wstack": wstack,
            "w2rep": w2rep, "dw16": dw16h, "ident": ident,
        })
    return in_maps, ze_flat


def postprocess(results, ze_flat):
    rs = np.concatenate([r["out_rs"] for r in results], axis=0)
    zdl = np.concatenate([r["out_zdl"] for r in results], axis=0)

    diff = (zdl - ze_flat).astype(np.float32)
    e_latent = np.mean(diff.astype(np.float64) ** 2)
    loss = np.float32(BETA * e_latent)

    z_st = (ze_flat + diff).reshape(B, HW, HW, DIM)
    z_st = np.ascontiguousarray(np.transpose(z_st, (0, 3, 1, 2)))

    counts = np.count_nonzero(rs, axis=0).astype(np.float64)
    avg = counts / counts.sum()
    perp = np.float32(np.exp(-np.sum(avg * np.log(avg + EPS))))
    return loss, z_st, perp, rs


def kernel(z_e, dict_w, rep_w, rep_b):
    from concourse.bass_utils import run_bass_kernel_spmd
    nc = _get_nc()
    in_maps, ze_flat = prepare_in_maps(z_e, dict_w, rep_w, rep_b)
    last_err = None
    for _attempt in range(3):
        try:
            res = run_bass_kernel_spmd(nc, in_maps, list(range(NCORES)))
            return postprocess(res.results, ze_flat)
        except Exception as e:  # transient device errors: retry
            last_err = e
            import time
            time.sleep(5)
    raise last_err


if __name__ == "__main__":
    rng = np.random.default_rng(0)
    z_e = rng.standard_normal((B, DIM, HW, HW)).astype(np.float32)
    dict_w = rng.standard_normal((K, DIM)).astype(np.float32)
    rep_w = (rng.standard_normal((K, DIM)) / np.sqrt(DIM)).astype(np.float32)
    rep_b = np.zeros((K,), dtype=np.float32)
    out = kernel(z_e, dict_w, rep_w, rep_b)
    print("loss", out[0], "perp", out[2], "z_st", out[1].shape, "rs", out[3].shape)


# revision 22
# speedup vs baseline: 1.0267x; 1.0267x over previous
"""VQ dictionary-learning forward kernel for Trainium2 (8 NeuronCores).

Per 128-token tile (tokens on partitions):
  - logits = x @ rep_w.T            : fp16 matmul (smooth path)
  - s2     = x @ dict_w.T - w2/2    : fp16 hi/lo 3-term split => exact
                                      (~1e-6 abs; selection-critical),
                                      w2 folded in as a c=3 row-stack matmul
  - rep    = softmax over K=1024    : exp+rowsum on ACT, scale on DVE
  - top-8  : vector.max (max8) over s2 PSUM; threshold = 8th value
  - rep_sparse = (s2 >= t8) * rep/8 : one fused scalar_tensor_tensor
  - z_dl   = rep_sparse @ dict_w    : fp16 PE transposes + fp16 matmul
  - loss / perplexity / straight-through / NCHW: host side (scalar stats)
Sharding: data-parallel over 32768 tokens, 4096/core; weights replicated.
"""
import sys

sys.path.insert(0, '/opt/trn_rl_repo')

import numpy as np

DIM = 256
K = 1024
SP = 8
BETA = 0.25
EPS = 1e-10
B = 32
HW = 32
N = B * HW * HW          # 32768 tokens
NCORES = 8
NSH = N // NCORES        # 4096 tokens per core
NT = NSH // 128          # 32 tiles per core
import os as _os
_KNT = int(_os.environ.get("KNT", NT))
_SKIP_ZDL = _os.environ.get("SKIP_ZDL", "0") == "1"
_SKIP_W2 = _os.environ.get("SKIP_W2", "0") == "1"
_SKIP_MAXSTT = _os.environ.get("SKIP_MAXSTT", "0") == "1"

KB = K // 512            # 2 k-chunks of 512


def _split16(a):
    """fp32 -> (hi, lo) fp16 pair with hi + lo ~= a to ~2^-22."""
    hi = a.astype(np.float16)
    lo = (a - hi.astype(np.float32)).astype(np.float16)
    return np.ascontiguousarray(hi), np.ascontiguousarray(lo)


def build_nc():
    import concourse.bacc as bacc
    import concourse.mybir as mybir
    import concourse.tile as tile

    f32 = mybir.dt.float32
    f16 = mybir.dt.float16
    ACT = mybir.ActivationFunctionType
    ALU = mybir.AluOpType

    nc = bacc.Bacc(None, target_bir_lowering=False)

    # ---- DRAM I/O ----
    xhlT = nc.dram_tensor("xhlT", [2, DIM, NSH], f16, kind="ExternalInput")
    wstack = nc.dram_tensor("wstack", [3, 2, 128, K], f16, kind="ExternalInput")
    w2rep = nc.dram_tensor("w2rep", [128, K], f32, kind="ExternalInput")
    dw16 = nc.dram_tensor("dw16", [K, DIM], f16, kind="ExternalInput")
    ident = nc.dram_tensor("ident", [128, 128], f16, kind="ExternalInput")
    out_rs = nc.dram_tensor("out_rs", [NSH, K], f32, kind="ExternalOutput")
    out_zdl = nc.dram_tensor("out_zdl", [NSH, DIM], f32, kind="ExternalOutput")

    with tile.TileContext(nc) as tc:
        with (
            tc.tile_pool(name="wts", bufs=1) as wts,
            tc.tile_pool(name="io", bufs=3) as io,
            tc.tile_pool(name="wk", bufs=2) as wk,
            tc.tile_pool(name="pslg", bufs=1, space="PSUM") as pslg,
            tc.tile_pool(name="pss2", bufs=2, space="PSUM") as pss2,
            tc.tile_pool(name="pso", bufs=1, space="PSUM") as pso,
        ):
            # ---- prefetch x tiles 0/1 before weights on the sync queue ----
            xhl_pre = []
            for i in range(2):
                t = io.tile([128, 2, 2, 128], f16, name=f"xhl_{i}", tag="xhl")
                nc.sync.dma_start(
                    t[:], xhlT[:, :, i * 128:(i + 1) * 128].rearrange(
                        "hl (c p) t -> p hl c t", p=128))
                xhl_pre.append(t)

            # ---- persistent weights: one big stacked tile, 2 parallel DMAs ----
            W_sb = wts.tile([128, 3, 2, K], f16, name="W_sb", tag="W_sb")
            nc.sync.dma_start(
                W_sb[:, :, 0, :], wstack[:, 0].rearrange("w p k -> p w k"))
            nc.scalar.dma_start(
                W_sb[:, :, 1, :], wstack[:, 1].rearrange("w p k -> p w k"))
            wh_sb = [W_sb[:, 0, c] for c in range(2)]
            wl_sb = [W_sb[:, 1, c] for c in range(2)]
            rw_sb = [W_sb[:, 2, c] for c in range(2)]
            w2_sb = wts.tile([128, K], f32, name="w2sb", tag="w2sb")
            nc.scalar.dma_start(w2_sb[:], w2rep[:])
            id_sb = wts.tile([128, 128], f16, name="idsb", tag="idsb")
            nc.scalar.dma_start(id_sb[:], ident[:])
            dw_sb = wts.tile([128, 8, DIM], f16, name="dwsb", tag="dwsb")
            nc.scalar.dma_start(dw_sb[:], dw16[:].rearrange("(j p) d -> p j d", p=128))

            # ---- PE warm-up: HAM un-throttles after ~3.4us of activity.
            # These depend only on a memset, so they run during the initial
            # weight/x DMAs which would otherwise leave the PE cold.
            wu_sb = wts.tile([128, 512], f16, name="wu_sb", tag="wu")
            nc.vector.memset(wu_sb[:], 0.0)
            wu_ps = pso.tile([128, 512], f32, name="wu_ps", tag="tp")
            for wi in range(30):
                nc.tensor.matmul(wu_ps[:], wu_sb[:, 0:128], wu_sb[:],
                                 start=(wi == 0), stop=(wi == 29))

            def emit_transpose(pi, rs16_sb):
                rsT_sb = wk.tile([128, K], f16, name=f"rsT_{pi}", tag="rsT")
                tp_ps = pso.tile([128, K], f16, name=f"tp_{pi}", tag="tp")
                for jj in range(8):
                    nc.tensor.transpose(
                        tp_ps[:, jj * 128:(jj + 1) * 128],
                        rs16_sb[:, jj * 128:(jj + 1) * 128], id_sb[:])
                return (pi, rsT_sb, tp_ps)

            def emit_rst_copies(pi, rsT_sb, tp_ps):
                for half in range(2):
                    nc.vector.tensor_copy(rsT_sb[:, half * 512:(half + 1) * 512],
                                          tp_ps[:, half * 512:(half + 1) * 512])

            def emit_zdl(pi, rsT_sb, tp_ps):
                psel = slice(pi * 128, pi * 128 + 128)
                zdl_ps = pso.tile([128, DIM], f32, name=f"zdl_{pi}", tag="zdl")
                for j in range(8):
                    nc.tensor.matmul(zdl_ps[:], rsT_sb[:, j * 128:(j + 1) * 128],
                                     dw_sb[:, j, :],
                                     start=(j == 0), stop=(j == 7))
                zdl_sb = io.tile([128, DIM], f32, name=f"zdl_sb_{pi}", tag="zdlsb")
                nc.scalar.activation(zdl_sb[:], zdl_ps[:], ACT.Copy)
                nc.sync.dma_start(out_zdl[psel, :], zdl_sb[:])

            prev = None
            prev2 = None
            for i in range(_KNT):
                sel = slice(i * 128, i * 128 + 128)
                # ---- load x tile: one DMA for hi+lo, both 128-d chunks ----
                if i < 2:
                    xhl = xhl_pre[i]
                else:
                    xhl = io.tile([128, 2, 2, 128], f16, name=f"xhl_{i}", tag="xhl")
                    nc.sync.dma_start(
                        xhl[:], xhlT[:, :, sel].rearrange("hl (c p) t -> p hl c t", p=128))
                xh = xhl[:, 0]
                xl = xhl[:, 1]

                # ---- PE: logits first (unblocks ACT early), then s2 ----
                lg_ps = pslg.tile([128, K], f32, name=f"lg_{i}", tag="lg")
                s2_ps = pss2.tile([128, K], f32, name=f"s2_{i}", tag="s2")
                for kb in range(KB):
                    kbs = slice(kb * 512, kb * 512 + 512)
                    for c in range(2):
                        nc.tensor.matmul(lg_ps[:, kbs], xh[:, c], rw_sb[c][:, kbs],
                                         start=(c == 0), stop=(c == 1))
                for kb in range(KB):
                    kbs = slice(kb * 512, kb * 512 + 512)
                    for c in range(2):
                        nc.tensor.matmul(s2_ps[:, kbs], xh[:, c], wh_sb[c][:, kbs],
                                         start=(c == 0), stop=False)
                        nc.tensor.matmul(s2_ps[:, kbs], xh[:, c], wl_sb[c][:, kbs],
                                         start=False, stop=False)
                        nc.tensor.matmul(s2_ps[:, kbs], xl[:, c], wh_sb[c][:, kbs],
                                         start=False, stop=(c == 1))

                nc.vector.tensor_tensor(s2_ps[:], s2_ps[:], w2_sb[:],
                                        op=ALU.add)

                tstage = None
                if prev is not None:
                    tstage = emit_transpose(*prev)
                    prev = None
                if prev2 is not None:
                    emit_zdl(*prev2)
                    prev2 = None

                # ---- softmax + top-8 + fused sparse-mask ----
                e_sb = wk.tile([128, K], f32, name=f"e_{i}", tag="e")
                sum_sb = wk.tile([128, 1], f32, name=f"sum_{i}", tag="sum")
                nc.scalar.activation(e_sb[:], lg_ps[:], ACT.Exp,
                                     accum_out=sum_sb[:])
                rinv_sb = wk.tile([128, 1], f32, name=f"rinv_{i}", tag="rinv")
                nc.vector.reciprocal(rinv_sb[:], sum_sb[:])
                c8_sb = wk.tile([128, 1], f32, name=f"c8_{i}", tag="c8")
                nc.vector.tensor_scalar_mul(c8_sb[:], rinv_sb[:], 0.125)
                top8_sb = wk.tile([128, 8], f32, name=f"top8_{i}", tag="top8")
                rsraw_sb = wk.tile([128, K], f32, name=f"rsraw_{i}", tag="rsraw")
                nc.vector.max(top8_sb[:], s2_ps[:])
                nc.vector.scalar_tensor_tensor(
                    rsraw_sb[:], s2_ps[:], top8_sb[:, 7:8], e_sb[:],
                    op0=ALU.is_ge, op1=ALU.mult)
                rs_sb = io.tile([128, K], f32, name=f"rs_{i}", tag="rs")
                nc.scalar.activation(rs_sb[:], rsraw_sb[:], ACT.Copy,
                                     scale=c8_sb[:])
                nc.sync.dma_start(out_rs[sel, :], rs_sb[:])
                rs16_sb = wk.tile([128, K], f16, name=f"rs16_{i}", tag="rs16")
                nc.scalar.activation(rs16_sb[:], rsraw_sb[:], ACT.Copy,
                                     scale=c8_sb[:])
                prev = (i, rs16_sb)
                if tstage is not None:
                    emit_rst_copies(*tstage)
                    prev2 = tstage

            for wi in range(10):
                nc.tensor.matmul(wu_ps[:], wu_sb[:, 0:128], wu_sb[:],
                                 start=(wi == 0), stop=(wi == 9),
                                 skip_group_check=True)
            if prev2 is not None:
                emit_zdl(*prev2)
            if prev is not None:
                tstage = emit_transpose(*prev)
                emit_rst_copies(*tstage)
                emit_zdl(*tstage)

    nc.compile()
    return nc


_NC_CACHE = {}


def _get_nc():
    if "nc" not in _NC_CACHE:
        _NC_CACHE["nc"] = build_nc()
    return _NC_CACHE["nc"]


def prepare_in_maps(z_e, dict_w, rep_w, rep_b):
    z_e = np.asarray(z_e, dtype=np.float32)
    dict_w = np.asarray(dict_w, dtype=np.float32)
    rep_w = np.asarray(rep_w, dtype=np.float32)

    ze_flat = np.ascontiguousarray(
        np.transpose(z_e, (0, 2, 3, 1)).reshape(N, DIM))

    whT, wlT = _split16(np.ascontiguousarray(dict_w.T))
    rwT = np.ascontiguousarray(rep_w.T).astype(np.float16)
    wstack = np.ascontiguousarray(np.stack([whT, wlT, rwT]).reshape(3, 2, 128, K))
    # exact -w2/2 as a 3-term fp16 sum
    w2 = (dict_w.astype(np.float64) ** 2).sum(1)
    v = (-0.5 * w2).astype(np.float32)
    w2rep = np.ascontiguousarray(np.broadcast_to(v, (128, K)).copy())
    ident = np.eye(128, dtype=np.float16)
    dw16h = dict_w.astype(np.float16)

    in_maps = []
    for c in range(NCORES):
        xT = np.ascontiguousarray(ze_flat[c * NSH:(c + 1) * NSH].T)
        xhT, xlT_ = _split16(xT)
        xhl = np.ascontiguousarray(np.stack([xhT, xlT_]))
        in_maps.append({
            "xhlT": xhl,
            "wstack": wstack,
            "w2rep": w2rep, "dw16": dw16h, "ident": ident,
        })
    return in_maps, ze_flat


def postprocess(results, ze_flat):
    rs = np.concatenate([r["out_rs"] for r in results], axis=0)
    zdl = np.concatenate([r["out_zdl"] for r in results], axis=0)

    diff = (zdl - ze_flat).astype(np.float32)
    e_latent = np.mean(diff.astype(np.float64) ** 2)
    loss = np.float32(BETA * e_latent)

    z_st = (ze_flat + diff).reshape(B, HW, HW, DIM)
    z_st = np.ascontiguousarray(np.transpose(z_st, (0, 3, 1, 2)))

    counts = np.count_nonzero(rs, axis=0).astype(np.float64)
    avg = counts / counts.sum()
    perp = np.float32(np.exp(-np.sum(avg * np.log(avg + EPS))))
    return loss, z_st, perp, rs


def kernel(z_e, dict_w, rep_w, rep_b):
    from concourse.bass_utils import run_bass_kernel_spmd
    nc = _get_nc()
    in_maps, ze_flat = prepare_in_maps(z_e, dict_w, rep_w, rep_b)
    last_err = None
    for _attempt in range(3):
        try:
            res = run_bass_kernel_spmd(nc, in_maps, list(range(NCORES)))
            return postprocess(res.results, ze_flat)
        except Exception as e:  # transient device errors: retry
            last_err = e
            import time
            time.sleep(5)
    raise last_err


if __name__ == "__main__":
    rng = np.random.default_rng(0)
    z_e = rng.standard_normal((B, DIM, HW, HW)).astype(np.float32)
    dict_w = rng.standard_normal((K, DIM)).astype(np.float32)
    rep_w = (rng.standard_normal((K, DIM)) / np.sqrt(DIM)).astype(np.float32)
    rep_b = np.zeros((K,), dtype=np.float32)
    out = kernel(z_e, dict_w, rep_w, rep_b)
    print("loss", out[0], "perp", out[2], "z_st", out[1].shape, "rs", out[3].shape)
